# revision 11
# baseline (speedup 1.0000x reference)
"""Self-contained Trainium2 Bass kernel for a 2-layer GAT + BatchNorm + graph pooling.

Contract: kernel(**inputs) takes the FULL (unsharded) inputs and returns the
FULL [G, 1024] float32 output. Internally: shard nodes (and their incident
edges) across 8 NeuronCores, run one SPMD Bass program (AllGather for the
inter-layer feature table, AllReduce for BatchNorm stats), then combine the
per-core partial poolings on the host.

Algorithm mapping (per core, own = 1/8 contiguous slice of nodes):
  dense L1: T1_own rows [h1 | al_src1 | al_dst1] = x_own @ [W1 | B1]
  AllGather T1 -> full gather table
  edge L1:  per 128-dst-node block, dma_gather rows of T1 by src (split into
            two index streams because dma_gather indices are int16),
            softmax-weighted segment sum via 0/1 selector-matrix matmuls
            accumulating [out | denom] in PSUM, epilogue divides by denom.
            (The segment-max subtraction of the reference softmax is dropped:
            alpha = ex/sum(ex) is mathematically identical and logits are
            O(+-8) so fp32 exp is safe.)
  BN stats partial sums -> AllReduce -> scale/shift; x1 = relu(bn(g1))
  dense L2 / AllGather T2 / edge L2 (+relu) analogous.
  pooling:  per-channel-tile segmented running sum & max along the node axis
            (tensor_tensor_scan with mask resets at graph starts); host reads
            the scan value at each graph's last column (pure indexing) and
            adds/maxes the <=2 per-graph partials from adjacent cores.
"""

import numpy as np

import concourse.bass as bass
import concourse.bacc as bacc
import concourse.tile as tile
from concourse import mybir
from concourse import bass_utils
from concourse.masks import make_identity

F32 = mybir.dt.float32
I16 = mybir.dt.int16
ALU = mybir.AluOpType
ACTF = mybir.ActivationFunctionType

# problem constants (hardcoded per the harness contract)
N, F_IN, C0, C1, H, E, G = 50000, 128, 64, 64, 4, 800000, 256
DEBUG = False
PHASES = 6   # build phases 1..PHASES (bisection aid): 1=denseL1+AG, 2=+edgeL1,
             # 3=+BN/AR, 4=+denseL2+AG, 5=+edgeL2, 6=+pooling
NB_LIMIT = None   # bisection aid: process only first n dst-blocks in edge phases
EDGE_OPS = 3      # bisection aid: 1=gathers only, 2=+DVE/ACT prep, 3=full
HC = H * C0            # 256
NEG_SLOPE = 0.2
BN_EPS = 1e-5
NCORES = 8
NPC = N // NCORES      # nodes per core (6250)
SPLIT = 32768          # dma_gather int16 index limit -> split gather table
RW = 320               # gather-table row width in f32 (1280 B, mult of 256 B)
ALW = 64               # al_dst table row width in f32 (256 B)
PART = 128


# --------------------------------------------------------------------------
# host-side preprocessing
# --------------------------------------------------------------------------

def _pack16(stream_i16, ncols):
    """dma_gather index layout: position i -> [i%16, i//16], replicated to
    partition groups 16k+p for the 8 Q7 cores."""
    base = stream_i16.reshape(ncols, 16).T          # [16, ncols]
    return np.tile(base, (8, 1)).astype(np.int16)   # [128, ncols]


def preprocess(x, edge_index, batch,
               W1, att_src1, att_dst1, b1, gamma, beta,
               W2, att_src2, att_dst2, b2):
    x = np.asarray(x, np.float32)
    edge_index = np.asarray(edge_index)
    batch = np.asarray(batch).astype(np.int64)
    W1 = np.asarray(W1, np.float32); W2 = np.asarray(W2, np.float32)

    src = np.concatenate([edge_index[0], np.arange(N, dtype=np.int64)])
    dst = np.concatenate([edge_index[1], np.arange(N, dtype=np.int64)])

    NB = (NPC + PART - 1) // PART                      # dst blocks per core

    per_core = [(dst >= r * NPC) & (dst < (r + 1) * NPC) for r in range(NCORES)]

    blocks = []     # blocks[r][b] = (lo_src, hi_src, within, dloc) arrays
    nlo_max = nhi_max = 0
    for r in range(NCORES):
        m = per_core[r]
        s_r = src[m]
        dloc = (dst[m] - r * NPC)
        order = np.argsort(dloc, kind="stable")
        s_r = s_r[order]; dloc = dloc[order]
        blk = dloc // PART
        core_blocks = []
        for b in range(NB):
            bm = blk == b
            sb_ = s_r[bm]; db_ = dloc[bm]
            lo_m = sb_ < SPLIT
            lo_src = sb_[lo_m]; hi_src = sb_[~lo_m] - SPLIT
            within = (db_ % PART).astype(np.float32)
            core_blocks.append((lo_src, hi_src, within[lo_m], within[~lo_m],
                                db_[lo_m], db_[~lo_m]))
            nlo_max = max(nlo_max, len(lo_src))
            nhi_max = max(nhi_max, len(hi_src))
        blocks.append(core_blocks)

    KLO = max(1, (nlo_max + PART - 1) // PART)
    KHI = max(1, (nhi_max + PART - 1) // PART)
    KT = KLO + KHI

    idx_lo = np.zeros((NCORES, NB, PART, KLO * 8), np.int16)
    idx_hi = np.zeros((NCORES, NB, PART, KHI * 8), np.int16)
    idx_al = np.zeros((NCORES, NB, PART, KT * 8), np.int16)
    dstloc = np.full((NCORES, NB, PART, KT), 999.0, np.float32)
    for r in range(NCORES):
        for b in range(NB):
            lo_src, hi_src, w_lo, w_hi, al_lo, al_hi = blocks[r][b]
            ls = np.zeros(KLO * PART, np.int16); ls[:len(lo_src)] = lo_src
            hs = np.zeros(KHI * PART, np.int16); hs[:len(hi_src)] = hi_src
            als = np.zeros(KT * PART, np.int16)
            als[:len(al_lo)] = al_lo
            als[KLO * PART:KLO * PART + len(al_hi)] = al_hi
            ds = np.full(KT * PART, 999.0, np.float32)
            ds[:len(w_lo)] = w_lo
            ds[KLO * PART:KLO * PART + len(w_hi)] = w_hi
            idx_lo[r, b] = _pack16(ls, KLO * 8)
            idx_hi[r, b] = _pack16(hs, KHI * 8)
            idx_al[r, b] = _pack16(als, KT * 8)
            dstloc[r, b] = ds.reshape(KT, PART).T

    # batch-derived pooling metadata
    counts = np.bincount(batch, minlength=G).astype(np.float64)
    maskrow = np.zeros((NCORES, 1, NPC), np.float32)
    cinvrow = np.zeros((NCORES, 1, NPC), np.float32)
    lastcol = [dict() for _ in range(NCORES)]  # graph -> last own column
    for r in range(NCORES):
        bseg = batch[r * NPC:(r + 1) * NPC]
        same = np.ones(NPC, np.float32)
        same[0] = 0.0
        same[1:] = (bseg[1:] == bseg[:-1]).astype(np.float32)
        maskrow[r, 0] = same
        cinvrow[r, 0] = (1.0 / np.maximum(counts[bseg], 1.0)).astype(np.float32)
        # last occurrence of each graph id in this core's slice
        gids, last_idx = np.unique(bseg[::-1], return_index=True)
        for g_, li in zip(gids, last_idx):
            lastcol[r][int(g_)] = NPC - 1 - int(li)

    # weight preprocessing (pure functions of weight inputs)
    def bmat(W, a_s, a_d, fin):
        Wr = W.reshape(fin, H, C0)
        bs = np.einsum("khc,hc->kh", Wr, np.asarray(a_s, np.float32))
        bd = np.einsum("khc,hc->kh", Wr, np.asarray(a_d, np.float32))
        return np.concatenate([bs, bd], axis=1).astype(np.float32)  # [fin, 8]

    B1 = bmat(W1, att_src1, att_dst1, F_IN)
    B2 = bmat(W2, att_src2, att_dst2, HC)

    shared = dict(
        W1=W1, B1=B1, W2=W2, B2=B2,
        b1row=np.asarray(b1, np.float32).reshape(2, PART),
        b2row=np.asarray(b2, np.float32).reshape(2, PART),
        gcol=np.asarray(gamma, np.float32).reshape(2, PART).T.copy(),
        bcol=np.asarray(beta, np.float32).reshape(2, PART).T.copy(),
    )
    in_maps = []
    for r in range(NCORES):
        in_maps.append(dict(
            shared,
            xT=np.ascontiguousarray(x[r * NPC:(r + 1) * NPC].T),
            idx_lo=idx_lo[r].reshape(NB, PART, KLO * 8),
            idx_hi=idx_hi[r].reshape(NB, PART, KHI * 8),
            idx_al=idx_al[r].reshape(NB, PART, KT * 8),
            dstloc=dstloc[r],
            maskrow=maskrow[r],
            cinvrow=cinvrow[r],
        ))
    meta = dict(NB=NB, KLO=KLO, KHI=KHI, KT=KT, lastcol=lastcol, counts=counts)
    return in_maps, meta


# --------------------------------------------------------------------------
# device program
# --------------------------------------------------------------------------

def build_program(meta):
    NB, KLO, KHI, KT = meta["NB"], meta["KLO"], meta["KHI"], meta["KT"]
    nc = bacc.Bacc("TRN2", target_bir_lowering=False, debug=False,
                   num_devices=NCORES)

    def ein(name, shape, dt=F32):
        return nc.dram_tensor(name, list(shape), dt, kind="ExternalInput").ap()

    xT_d = ein("xT", [F_IN, NPC])
    W1_d = ein("W1", [F_IN, HC]);  B1_d = ein("B1", [F_IN, 8])
    W2_d = ein("W2", [HC, HC]);    B2_d = ein("B2", [HC, 8])
    b1r_d = ein("b1row", [2, PART]); b2r_d = ein("b2row", [2, PART])
    gcol_d = ein("gcol", [PART, 2]); bcol_d = ein("bcol", [PART, 2])
    ilo_d = ein("idx_lo", [NB, PART, KLO * 8], I16)
    ihi_d = ein("idx_hi", [NB, PART, KHI * 8], I16)
    ial_d = ein("idx_al", [NB, PART, KT * 8], I16)
    dl_d = ein("dstloc", [NB, PART, KT])
    mask_d = ein("maskrow", [1, NPC])
    cinv_d = ein("cinvrow", [1, NPC])

    omax_d = nc.dram_tensor("out_max", [4 * PART, NPC], F32, kind="ExternalOutput").ap()
    omean_d = nc.dram_tensor("out_mean", [4 * PART, NPC], F32, kind="ExternalOutput").ap()
    if DEBUG:
        dT1_d = nc.dram_tensor("d_T1", [N, 264], F32, kind="ExternalOutput").ap()
        dT2_d = nc.dram_tensor("d_T2", [N, 264], F32, kind="ExternalOutput").ap()
        dbn_d = nc.dram_tensor("d_bn", [PART, 4], F32, kind="ExternalOutput").ap()
        dg1T_d = nc.dram_tensor("d_g1T", [HC, NPC], F32, kind="ExternalOutput").ap()
        dx2T_d = nc.dram_tensor("d_x2T", [HC, NPC], F32, kind="ExternalOutput").ap()
        dar_d = nc.dram_tensor("d_ar", [PART, 4], F32, kind="ExternalOutput").ap()

    # internal DRAM
    ag1_in = nc.dram_tensor("ag1_in", [NPC, RW], F32).ap()
    T1 = nc.dram_tensor("T1", [N, RW], F32, addr_space="Shared").ap()
    ag2_in = nc.dram_tensor("ag2_in", [NPC, RW], F32).ap()
    T2 = nc.dram_tensor("T2", [N, RW], F32, addr_space="Shared").ap()
    al1 = nc.dram_tensor("al1", [NB * PART, ALW], F32).ap()
    al2 = nc.dram_tensor("al2", [NB * PART, ALW], F32).ap()
    g1T = nc.dram_tensor("g1T", [HC, NPC], F32).ap()
    x2T = nc.dram_tensor("x2T", [HC, NPC], F32).ap()
    ar_in = nc.dram_tensor("ar_in", [PART, 4], F32).ap()
    ar_out = nc.dram_tensor("ar_out", [PART, 4], F32, addr_space="Shared").ap()

    rgroups = [list(range(NCORES))]

    class _PhaseStopE(Exception):
        pass
    global _PhaseStop
    _PhaseStop = _PhaseStopE

    with tile.TileContext(nc) as tc:
      try:
        # ---------- shared constant tiles ----------
        with tc.tile_pool(name="const", bufs=1) as cpool:
            ident = cpool.tile([PART, PART], F32)
            make_identity(nc, ident[:])
            iota_i = cpool.tile([PART, PART], mybir.dt.int32)
            nc.gpsimd.iota(iota_i[:], pattern=[[1, PART]], base=0,
                           channel_multiplier=0)
            iota_f = cpool.tile([PART, PART], F32)
            nc.vector.tensor_copy(out=iota_f[:], in_=iota_i[:])

            def bias_bcast(row_d, pool, psum_pool, tag):
                bv = pool.tile([PART, 2], F32, tag=f"biasv{tag}")
                nc.sync.dma_start(out=bv[:], in_=row_d[:, :].rearrange("c p -> p c"))
                bb = pool.tile([PART, HC], F32, tag=f"biasb{tag}")
                for c in range(2):
                    tp = psum_pool.tile([PART, PART], F32, tag=f"biastp{tag}{c}")
                    nc.tensor.transpose(out=tp[:], in_=bv[:, c:c + 1].to_broadcast([PART, PART]),
                                        identity=ident[:])
                    nc.vector.tensor_copy(out=bb[:, c * PART:(c + 1) * PART], in_=tp[:])
                return bb

            with tc.tile_pool(name="biasps", bufs=1, space="PSUM") as bps:
                b1b = bias_bcast(b1r_d, cpool, bps, "1")
                b2b = bias_bcast(b2r_d, cpool, bps, "2")

            # ---------- dense L1 ----------
            _sc = nc.enter_named_scope("dense1", False)[0]
            with tc.tile_pool(name="d1", bufs=2) as dp, \
                 tc.tile_pool(name="d1w", bufs=1) as wp, \
                 tc.tile_pool(name="d1ps", bufs=2, space="PSUM") as pp:
                xT_sb = wp.tile([F_IN, NPC], F32)
                nc.sync.dma_start(out=xT_sb[:], in_=xT_d[:, :])
                W1_sb = wp.tile([F_IN, HC], F32)
                nc.sync.dma_start(out=W1_sb[:], in_=W1_d[:, :])
                B1_sb = wp.tile([F_IN, 8], F32)
                nc.sync.dma_start(out=B1_sb[:], in_=B1_d[:, :])
                for b in range(NB):
                    mb = min(PART, NPC - b * PART)
                    ps = pp.tile([PART, 264], F32, tag="dps")
                    nc.tensor.matmul(ps[0:mb, 0:HC], lhsT=xT_sb[:, b * PART:b * PART + mb],
                                     rhs=W1_sb[:], start=True, stop=True)
                    nc.tensor.matmul(ps[0:mb, HC:HC + 8], lhsT=xT_sb[:, b * PART:b * PART + mb],
                                     rhs=B1_sb[:], start=True, stop=True)
                    cp = dp.tile([PART, 264], F32, tag="dcp")
                    nc.vector.tensor_copy(out=cp[0:mb, :], in_=ps[0:mb, :])
                    nc.sync.dma_start(out=ag1_in[b * PART:b * PART + mb, 0:264],
                                      in_=cp[0:mb, 0:264])
                    nc.sync.dma_start(out=al1[b * PART:b * PART + mb, 0:4],
                                      in_=cp[0:mb, 260:264])

            nc.leave_named_scope("dense1", _sc, False)
            _sc = nc.enter_named_scope("ag1", False)[0]
            nc.gpsimd.collective_compute(
                "AllGather", ALU.bypass, replica_groups=rgroups,
                ins=[ag1_in[:, :]], outs=[T1[:, :]])
            nc.leave_named_scope("ag1", _sc, False)

            # ---------- edge phase (shared for both layers) ----------
            def edge_phase(Tbl, altab, bbias, outT, relu, lidx, hidx, aidx,
                           scope="edge"):
                _es = nc.enter_named_scope(scope, False)[0]
                with tc.tile_pool(name="eidx", bufs=2) as ip, \
                     tc.tile_pool(name="eg", bufs=2) as gp, \
                     tc.tile_pool(name="ew", bufs=2) as wp2, \
                     tc.tile_pool(name="eps", bufs=2, space="PSUM") as ep, \
                     tc.tile_pool(name="etps", bufs=2, space="PSUM") as tps:
                    for b in range(NB if NB_LIMIT is None else min(NB, NB_LIMIT)):
                        mb = min(PART, NPC - b * PART)
                        il = ip.tile([PART, KLO * 8], I16, tag="il")
                        nc.sync.dma_start(out=il[:], in_=lidx[b, :, :])
                        ih = ip.tile([PART, KHI * 8], I16, tag="ih")
                        nc.sync.dma_start(out=ih[:], in_=hidx[b, :, :])
                        ia = ip.tile([PART, KT * 8], I16, tag="ia")
                        nc.sync.dma_start(out=ia[:], in_=aidx[b, :, :])
                        dl = ip.tile([PART, KT], F32, tag="dl")
                        nc.sync.dma_start(out=dl[:], in_=dl_d[b, :, :])

                        # dma_gather is limited to 1024 indices per
                        # instruction (HW hang beyond that) -> chunk by 8
                        # 128-row blocks.
                        def gather_chunks(gtile, src, ixtile, ktot, elem):
                            for c0 in range(0, ktot, 8):
                                cnt = min(8, ktot - c0)
                                nc.gpsimd.dma_gather(
                                    out_ap=gtile[:, c0:c0 + cnt, :],
                                    in_ap=src, idxs_ap=ixtile[:, c0 * 8:(c0 + cnt) * 8],
                                    num_idxs=cnt * PART, num_idxs_reg=cnt * PART,
                                    elem_size=elem)

                        glo = gp.tile([PART, KLO, RW], F32, tag="glo")
                        gather_chunks(glo, Tbl[:, :], il, KLO, RW)
                        ghi = gp.tile([PART, KHI, RW], F32, tag="ghi")
                        gather_chunks(ghi, Tbl[SPLIT:N, :], ih, KHI, RW)
                        ga = gp.tile([PART, KT, ALW], F32, tag="ga")
                        gather_chunks(ga, altab[:, :], ia, KT, ALW)

                        if EDGE_OPS == 1:
                            tok = wp2.tile([PART, 1], F32, tag="tok")
                            nc.vector.tensor_reduce(out=tok[:], in_=glo[:, 0, 0:HC],
                                                    axis=mybir.AxisListType.X, op=ALU.add)
                            nc.vector.tensor_tensor(out=tok[:], in0=tok[:], in1=ghi[:, 0, 0:1], op=ALU.add)
                            nc.vector.tensor_tensor(out=tok[:], in0=tok[:], in1=ga[:, 0, 0:1], op=ALU.add)
                            nc.sync.dma_start(out=outT[0:PART, b:b + 1], in_=tok[:])
                            continue
                        S01 = wp2.tile([PART, KT, PART], F32, tag="S01")
                        for e0 in range(0, KT, 4):
                            cnt = min(4, KT - e0)
                            nc.vector.tensor_tensor(
                                out=S01[:, e0:e0 + cnt, :],
                                in0=dl[:, e0:e0 + cnt].unsqueeze(-1).to_broadcast([PART, cnt, PART]),
                                in1=iota_f[:].unsqueeze(1).to_broadcast([PART, cnt, PART]),
                                op=ALU.is_equal)

                        Z = wp2.tile([PART, KT, 4], F32, tag="Z")
                        nc.vector.tensor_tensor(out=Z[:, 0:KLO, :],
                                                in0=glo[:, :, HC:HC + 4],
                                                in1=ga[:, 0:KLO, 0:4], op=ALU.add)
                        nc.vector.tensor_tensor(out=Z[:, KLO:KT, :],
                                                in0=ghi[:, :, HC:HC + 4],
                                                in1=ga[:, KLO:KT, 0:4], op=ALU.add)
                        ZT = wp2.tile([PART, KT, 4], F32, tag="ZT")
                        nc.vector.tensor_scalar_mul(out=ZT[:], in0=Z[:], scalar1=NEG_SLOPE)
                        nc.vector.tensor_tensor(out=Z[:], in0=Z[:], in1=ZT[:], op=ALU.max)
                        EX = wp2.tile([PART, KT, 4], F32, tag="EX")
                        nc.scalar.activation(out=EX[:], in_=Z[:], func=ACTF.Exp)

                        Hp = wp2.tile([PART, KT, 260], F32, tag="Hp")
                        nc.vector.tensor_tensor(
                            out=Hp[:, 0:KLO, 0:HC].rearrange("p k (h c) -> p k h c", h=H),
                            in0=glo[:, :, 0:HC].rearrange("p k (h c) -> p k h c", h=H),
                            in1=EX[:, 0:KLO, :].unsqueeze(-1).to_broadcast([PART, KLO, H, C0]),
                            op=ALU.mult)
                        nc.vector.tensor_tensor(
                            out=Hp[:, KLO:KT, 0:HC].rearrange("p k (h c) -> p k h c", h=H),
                            in0=ghi[:, :, 0:HC].rearrange("p k (h c) -> p k h c", h=H),
                            in1=EX[:, KLO:KT, :].unsqueeze(-1).to_broadcast([PART, KHI, H, C0]),
                            op=ALU.mult)
                        nc.vector.tensor_copy(out=Hp[:, :, HC:HC + 4], in_=EX[:])

                        if EDGE_OPS == 2:
                            tok2 = wp2.tile([PART, 1], F32, tag="tok2")
                            nc.vector.tensor_reduce(out=tok2[:], in_=Hp[:, 0, :],
                                                    axis=mybir.AxisListType.X, op=ALU.add)
                            nc.vector.tensor_tensor(out=tok2[:], in0=tok2[:], in1=S01[:, 0, 0:1], op=ALU.add)
                            nc.sync.dma_start(out=outT[0:PART, b:b + 1], in_=tok2[:])
                            continue
                        acc = ep.tile([PART, 260], F32, tag="acc")
                        for e in range(KT):
                            nc.tensor.matmul(acc[:], lhsT=S01[:, e, :], rhs=Hp[:, e, :],
                                             start=(e == 0), stop=(e == KT - 1))

                        dn = wp2.tile([PART, 4], F32, tag="dn")
                        nc.vector.tensor_scalar_add(out=dn[:], in0=acc[:, HC:HC + 4],
                                                    scalar1=1e-16)
                        rec = wp2.tile([PART, 4], F32, tag="rec")
                        nc.vector.reciprocal(out=rec[:], in_=dn[:])
                        ob = wp2.tile([PART, HC], F32, tag="ob")
                        nc.vector.tensor_tensor(
                            out=ob[:].rearrange("p (h c) -> p h c", h=H),
                            in0=acc[:, 0:HC].rearrange("p (h c) -> p h c", h=H),
                            in1=rec[:].unsqueeze(-1).to_broadcast([PART, H, C0]),
                            op=ALU.mult)
                        nc.vector.tensor_tensor(out=ob[:], in0=ob[:], in1=bbias[:], op=ALU.add)
                        if relu:
                            nc.vector.tensor_scalar_max(out=ob[:], in0=ob[:], scalar1=0.0)
                        for c in range(2):
                            tp = tps.tile([PART, PART], F32, tag="ttp")
                            nc.tensor.transpose(out=tp[:], in_=ob[:, c * PART:(c + 1) * PART],
                                                identity=ident[:])
                            tsb = wp2.tile([PART, PART], F32, tag="tsb")
                            nc.vector.tensor_copy(out=tsb[:], in_=tp[:])
                            nc.sync.dma_start(
                                out=outT[c * PART:(c + 1) * PART, b * PART:b * PART + mb],
                                in_=tsb[:, 0:mb])
                nc.leave_named_scope(scope, _es, False)

            if PHASES >= 2:
                edge_phase(T1, al1, b1b, g1T, relu=False,
                           lidx=ilo_d, hidx=ihi_d, aidx=ial_d, scope="edge1")

            # ---------- BN stats + AllReduce ----------
            if PHASES < 3:
                raise _PhaseStop
            _sc = nc.enter_named_scope("bnstat", False)[0]
            with tc.tile_pool(name="st", bufs=1) as sp, \
                 tc.tile_pool(name="stw", bufs=1) as sw:
                stats = sw.tile([PART, 4], F32)
                for ct in range(2):
                    gt = sp.tile([PART, NPC], F32, tag="gt")
                    nc.sync.dma_start(out=gt[:], in_=g1T[ct * PART:(ct + 1) * PART, :])
                    nc.vector.tensor_reduce(out=stats[:, ct:ct + 1], in_=gt[:],
                                            axis=mybir.AxisListType.X, op=ALU.add)
                    sq = sp.tile([PART, NPC], F32, tag="sq")
                    nc.scalar.activation(out=sq[:], in_=gt[:], func=ACTF.Square)
                    nc.vector.tensor_reduce(out=stats[:, 2 + ct:3 + ct], in_=sq[:],
                                            axis=mybir.AxisListType.X, op=ALU.add)
                nc.sync.dma_start(out=ar_in[:, :], in_=stats[:])
            nc.leave_named_scope("bnstat", _sc, False)

            _sc = nc.enter_named_scope("ar", False)[0]
            nc.gpsimd.collective_compute(
                "AllReduce", ALU.add, replica_groups=rgroups,
                ins=[ar_in[:, :]], outs=[ar_out[:, :]])
            nc.leave_named_scope("ar", _sc, False)

            with tc.tile_pool(name="bnw", bufs=1) as bw:
                ar_sb = bw.tile([PART, 4], F32)
                nc.sync.dma_start(out=ar_sb[:], in_=ar_out[:, :])
                mean = bw.tile([PART, 2], F32)
                nc.vector.tensor_scalar_mul(out=mean[:], in0=ar_sb[:, 0:2], scalar1=1.0 / N)
                msq = bw.tile([PART, 2], F32)
                nc.vector.tensor_scalar_mul(out=msq[:], in0=ar_sb[:, 2:4], scalar1=1.0 / N)
                var = bw.tile([PART, 2], F32)
                nc.vector.tensor_tensor(out=var[:], in0=mean[:], in1=mean[:], op=ALU.mult)
                nc.vector.tensor_tensor(out=var[:], in0=msq[:], in1=var[:], op=ALU.subtract)
                nc.vector.tensor_scalar_add(out=var[:], in0=var[:], scalar1=BN_EPS)
                sd = bw.tile([PART, 2], F32)
                nc.scalar.activation(out=sd[:], in_=var[:], func=ACTF.Sqrt)
                rinv = bw.tile([PART, 2], F32)
                nc.vector.reciprocal(out=rinv[:], in_=sd[:])
                gc = bw.tile([PART, 2], F32)
                nc.sync.dma_start(out=gc[:], in_=gcol_d[:, :])
                bc = bw.tile([PART, 2], F32)
                nc.sync.dma_start(out=bc[:], in_=bcol_d[:, :])
                scale_c = bw.tile([PART, 2], F32)
                nc.vector.tensor_tensor(out=scale_c[:], in0=gc[:], in1=rinv[:], op=ALU.mult)
                shift_c = bw.tile([PART, 2], F32)
                nc.vector.tensor_tensor(out=shift_c[:], in0=mean[:], in1=scale_c[:], op=ALU.mult)
                nc.vector.tensor_tensor(out=shift_c[:], in0=bc[:], in1=shift_c[:], op=ALU.subtract)

                # ---------- dense L2 ----------
                if PHASES < 4:
                    raise _PhaseStop
                _sc = nc.enter_named_scope("dense2", False)[0]
                with tc.tile_pool(name="d2", bufs=2) as dp2, \
                     tc.tile_pool(name="d2w", bufs=1) as wp3, \
                     tc.tile_pool(name="d2ps", bufs=2, space="PSUM") as pp2:
                    W2_sb = [wp3.tile([PART, HC], F32, tag=f"w2_{kt}", name=f"w2_{kt}")
                             for kt in range(2)]
                    B2_sb = [wp3.tile([PART, 8], F32, tag=f"b2_{kt}", name=f"b2_{kt}")
                             for kt in range(2)]
                    for kt in range(2):
                        nc.sync.dma_start(out=W2_sb[kt][:],
                                          in_=W2_d[kt * PART:(kt + 1) * PART, :])
                        nc.sync.dma_start(out=B2_sb[kt][:],
                                          in_=B2_d[kt * PART:(kt + 1) * PART, :])
                    for b in range(NB):
                        mb = min(PART, NPC - b * PART)
                        ps = pp2.tile([PART, 264], F32, tag="d2psacc")
                        x1s_l = []
                        for kt in range(2):
                            gsl = dp2.tile([PART, PART], F32, tag="gsl")
                            nc.sync.dma_start(
                                out=gsl[:, 0:mb],
                                in_=g1T[kt * PART:(kt + 1) * PART, b * PART:b * PART + mb])
                            x1s = dp2.tile([PART, PART], F32, tag="x1s")
                            nc.scalar.activation(out=x1s[:, 0:mb], in_=gsl[:, 0:mb],
                                                 func=ACTF.Relu,
                                                 bias=shift_c[:, kt:kt + 1],
                                                 scale=scale_c[:, kt:kt + 1])
                            x1s_l.append(x1s)
                        # NOTE: start=True clears has_written for the whole PSUM
                        # bank, so each region's accumulation group must finish
                        # before the next region starts.
                        for kt in range(2):
                            nc.tensor.matmul(ps[0:mb, 0:HC], lhsT=x1s_l[kt][:, 0:mb],
                                             rhs=W2_sb[kt][:],
                                             start=(kt == 0), stop=(kt == 1))
                        for kt in range(2):
                            nc.tensor.matmul(ps[0:mb, HC:HC + 8], lhsT=x1s_l[kt][:, 0:mb],
                                             rhs=B2_sb[kt][:],
                                             start=(kt == 0), stop=(kt == 1))
                        cp = dp2.tile([PART, 264], F32, tag="d2cp")
                        nc.vector.tensor_copy(out=cp[0:mb, :], in_=ps[0:mb, :])
                        nc.sync.dma_start(out=ag2_in[b * PART:b * PART + mb, 0:264],
                                          in_=cp[0:mb, 0:264])
                        nc.sync.dma_start(out=al2[b * PART:b * PART + mb, 0:4],
                                          in_=cp[0:mb, 260:264])

                nc.leave_named_scope("dense2", _sc, False)
                _sc = nc.enter_named_scope("ag2", False)[0]
                nc.gpsimd.collective_compute(
                    "AllGather", ALU.bypass, replica_groups=rgroups,
                    ins=[ag2_in[:, :]], outs=[T2[:, :]])
                nc.leave_named_scope("ag2", _sc, False)

                if PHASES < 5:
                    raise _PhaseStop
                edge_phase(T2, al2, b2b, x2T, relu=True,
                           lidx=ilo_d, hidx=ihi_d, aidx=ial_d, scope="edge2")

                if DEBUG and PHASES >= 6:
                    with tc.tile_pool(name="dbg", bufs=2) as dbp:
                        for rb in range(N // PART):
                            t_ = dbp.tile([PART, 264], F32, tag="dbgt")
                            nc.sync.dma_start(out=t_[:], in_=T1[rb * PART:(rb + 1) * PART, 0:264])
                            nc.sync.dma_start(out=dT1_d[rb * PART:(rb + 1) * PART, :], in_=t_[:])
                            t_b = dbp.tile([PART, 264], F32, tag="dbgtb")
                            nc.sync.dma_start(out=t_b[:], in_=T2[rb * PART:(rb + 1) * PART, 0:264])
                            nc.sync.dma_start(out=dT2_d[rb * PART:(rb + 1) * PART, :], in_=t_b[:])
                        t5_ = dbp.tile([PART, 4], F32, tag="dbg5")
                        nc.vector.tensor_copy(out=t5_[:, 0:2], in_=scale_c[:])
                        nc.vector.tensor_copy(out=t5_[:, 2:4], in_=shift_c[:])
                        nc.sync.dma_start(out=dbn_d[:, :], in_=t5_[:])
                        for ct in range(2):
                            t2_ = dbp.tile([PART, NPC], F32, tag="dbg2")
                            nc.sync.dma_start(out=t2_[:], in_=g1T[ct * PART:(ct + 1) * PART, :])
                            nc.sync.dma_start(out=dg1T_d[ct * PART:(ct + 1) * PART, :], in_=t2_[:])
                            t3_ = dbp.tile([PART, NPC], F32, tag="dbg3")
                            nc.sync.dma_start(out=t3_[:], in_=x2T[ct * PART:(ct + 1) * PART, :])
                            nc.sync.dma_start(out=dx2T_d[ct * PART:(ct + 1) * PART, :], in_=t3_[:])
                        t4_ = dbp.tile([PART, 4], F32, tag="dbg4")
                        nc.sync.dma_start(out=t4_[:], in_=ar_out[:, :])
                        nc.sync.dma_start(out=dar_d[:, :], in_=t4_[:])

                # ---------- pooling ----------
                if PHASES < 6:
                    raise _PhaseStop
                _sc = nc.enter_named_scope("pool", False)[0]
                with tc.tile_pool(name="pl", bufs=1) as pl:
                    mk = pl.tile([PART, NPC], F32, tag="mk")
                    nc.sync.dma_start(out=mk[:], in_=mask_d[0:1, :].to_broadcast([PART, NPC]))
                    cv = pl.tile([PART, NPC], F32, tag="cv")
                    nc.sync.dma_start(out=cv[:], in_=cinv_d[0:1, :].to_broadcast([PART, NPC]))
                    for ct in range(4):
                        xt = pl.tile([PART, NPC], F32, tag="xt")
                        if ct < 2:
                            gld = pl.tile([PART, NPC], F32, tag="gld")
                            nc.sync.dma_start(out=gld[:], in_=g1T[ct * PART:(ct + 1) * PART, :])
                            nc.scalar.activation(out=xt[:], in_=gld[:], func=ACTF.Relu,
                                                 bias=shift_c[:, ct:ct + 1],
                                                 scale=scale_c[:, ct:ct + 1])
                        else:
                            nc.sync.dma_start(out=xt[:],
                                              in_=x2T[(ct - 2) * PART:(ct - 1) * PART, :])
                        sm = pl.tile([PART, NPC], F32, tag="sm")
                        nc.vector.tensor_tensor_scan(out=sm[:], data0=mk[:], data1=xt[:],
                                                     initial=0.0, op0=ALU.mult, op1=ALU.max)
                        nc.sync.dma_start(out=omax_d[ct * PART:(ct + 1) * PART, :], in_=sm[:])
                        ss = pl.tile([PART, NPC], F32, tag="ss")
                        nc.vector.tensor_tensor_scan(out=ss[:], data0=mk[:], data1=xt[:],
                                                     initial=0.0, op0=ALU.mult, op1=ALU.add)
                        nc.vector.tensor_tensor(out=ss[:], in0=ss[:], in1=cv[:], op=ALU.mult)
                        nc.sync.dma_start(out=omean_d[ct * PART:(ct + 1) * PART, :], in_=ss[:])
                nc.leave_named_scope("pool", _sc, False)

      except _PhaseStopE:
        pass

    nc.compile()
    return nc


# --------------------------------------------------------------------------
# host-side combine
# --------------------------------------------------------------------------

def postprocess(results, meta):
    counts = meta["counts"]
    lastcol = meta["lastcol"]
    mean = np.zeros((G, 2 * HC), np.float32)
    mx = np.zeros((G, 2 * HC), np.float32)
    for r in range(NCORES):
        om = results[r]["out_mean"]   # [512, NPC]
        ox = results[r]["out_max"]
        for g_, col in lastcol[r].items():
            mean[g_] += om[:, col]
            mx[g_] = np.maximum(mx[g_], ox[:, col])
    # empty graphs stay 0 (matches reference semantics)
    return np.concatenate([mean, mx], axis=1).astype(np.float32)


_CACHE = {}


def kernel(**inputs):
    in_maps, meta = preprocess(**inputs)
    key = (meta["NB"], meta["KLO"], meta["KHI"])
    if key not in _CACHE:
        _CACHE[key] = build_program(meta)
    nc = _CACHE[key]
    res = bass_utils.run_bass_kernel_spmd(nc, in_maps, core_ids=list(range(NCORES)))
    return postprocess(res.results, meta)



# revision 20
# speedup vs baseline: 2.0906x; 2.0906x over previous
"""Self-contained Trainium2 Bass kernel for a 2-layer GAT + BatchNorm + graph pooling.

Contract: kernel(**inputs) takes the FULL (unsharded) inputs and returns the
FULL [G, 1024] float32 output.

v2 design (vs v1): replicated dense layers, fp16 gather tables, tiny collectives.
  - dense L1 is REPLICATED: every core computes the full table
    T1b[n] = [h(256 fp16) | al_src(4 f32, bitcast into 8 fp16 slots) | pad]
    (768 B rows) from x.  No AllGather of the 64 MB table (v1's main cost).
  - attention-logit "al_dst" values live in a 16-node-packed table
    alpk[n//16] = 4 f32 x 16 nodes (256 B rows, the dma_gather minimum), so
    the gather index fits int16 and the replicated write is small; the
    per-edge value is extracted with a one-hot dot on DVE.
  - edge phase (per 128-dst-node block): dma_gather rows by src (lo/hi split
    for int16), -1-padded index streams skip pad transfers (per-core valid
    counts come from an SBUF-loaded register), softmax-weighted segment sum
    via 0/1 fp16 selector-matrix matmuls accumulating [out | denom] in PSUM.
    Logits stay f32 (exp in f32, clamped at +8 before the fp16 cast so pad
    garbage cannot overflow fp16).
  - between layers only x1 (= relu(bn(g1)), 256ch fp16) is AllGathered —
    3.2 MB per rank in 4 column-chunks issued as edge L1 drains, so the wire
    time overlaps edge compute.  BN stats use a [128,4] AllReduce.
  - dense L2 is replicated from the gathered x1 chunks (BN affine + relu
    fused into the activation that loads each lhsT chunk).
  - pooling: per-channel-tile segmented running sum & max along the node axis
    (tensor_tensor_scan); host reads each graph's last column and combines
    the <=2 per-graph partials from adjacent cores.
"""

import numpy as np

import concourse.bass as bass
import concourse.bacc as bacc
import concourse.tile as tile
from concourse import mybir
from concourse import bass_utils
from concourse.masks import make_identity

F32 = mybir.dt.float32
F16 = mybir.dt.float16
I16 = mybir.dt.int16
I32 = mybir.dt.int32
ALU = mybir.AluOpType
ACTF = mybir.ActivationFunctionType

# problem constants (hardcoded per the harness contract)
N, F_IN, C0, C1, H, E, G = 50000, 128, 64, 64, 4, 800000, 256
HC = H * C0            # 256
NEG_SLOPE = 0.2
BN_EPS = 1e-5
NCORES = 8
NPC = N // NCORES      # nodes per core (6250)
SPLIT = 32768          # dma_gather int16 index limit -> split gather table
RWH = 384              # fp16 table row width (768 B): h(256) + al_src(8) + pad
PART = 128
NPAD = 50048           # N rounded to 128 blocks (391 blocks)
NBLK = NPAD // PART    # 391 dense blocks
CHUNK = 8              # 128-idx groups per dma_gather (1024 idx HW limit)
ZCLAMP = 8.0           # logit clamp (real logits ~ +-6); keeps exp fp16-finite
NAGC = 4               # AllGather column-chunks for the inter-layer feature

PHASES = 6             # build phases 1..6 (bisection aid)


# --------------------------------------------------------------------------
# host-side preprocessing
# --------------------------------------------------------------------------

def _pack16(stream_i16, ncols):
    """dma_gather index layout: position i -> [i%16, i//16], replicated to
    partition groups 16k+p for the 8 Q7 cores."""
    base = stream_i16.reshape(ncols, 16).T          # [16, ncols]
    return np.tile(base, (8, 1)).astype(np.int16)   # [128, ncols]


def _pad_stream(vals, nslots):
    """Pad an index stream to nslots with -1 (skipped by dma_gather) and
    return (idx_i16, per-1024-chunk valid counts).  A chunk with zero valid
    indices gets a single dummy index 0 (count 1): the interpreter/HW needs
    at least one non-negative index per instruction."""
    n = len(vals)
    out = np.full(nslots, -1, np.int16)
    out[:n] = vals
    counts = []
    for c0 in range(0, nslots, CHUNK * PART):
        span = min(CHUNK * PART, nslots - c0)
        cnt = min(max(n - c0, 0), span)
        if cnt == 0:
            out[c0] = 0
            cnt = 1
        counts.append(cnt)
    return out, counts


def preprocess(x, edge_index, batch,
               W1, att_src1, att_dst1, b1, gamma, beta,
               W2, att_src2, att_dst2, b2):
    x = np.asarray(x, np.float32)
    edge_index = np.asarray(edge_index)
    batch = np.asarray(batch).astype(np.int64)
    W1 = np.asarray(W1, np.float32); W2 = np.asarray(W2, np.float32)

    src = np.concatenate([edge_index[0], np.arange(N, dtype=np.int64)])
    dst = np.concatenate([edge_index[1], np.arange(N, dtype=np.int64)])

    NB = (NPC + PART - 1) // PART                      # dst blocks per core

    # ---- per-core edge streams ----
    blocks = []     # blocks[r][b] = (lo_src, hi_src, abs_lo, abs_hi)
    nlo_max = nhi_max = 0
    for r in range(NCORES):
        m = (dst >= r * NPC) & (dst < (r + 1) * NPC)
        s_r = src[m]
        d_r = dst[m]
        dloc = d_r - r * NPC
        order = np.argsort(dloc, kind="stable")
        s_r = s_r[order]; d_r = d_r[order]; dloc = dloc[order]
        blk = dloc // PART
        core_blocks = []
        for b in range(NB):
            bm = blk == b
            sb_ = s_r[bm]; db_ = d_r[bm]
            lo_m = sb_ < SPLIT
            core_blocks.append((sb_[lo_m], sb_[~lo_m] - SPLIT,
                                db_[lo_m], db_[~lo_m]))
            nlo_max = max(nlo_max, int(lo_m.sum()))
            nhi_max = max(nhi_max, int((~lo_m).sum()))
        blocks.append(core_blocks)

    KLO = max(1, (nlo_max + PART - 1) // PART)
    KHI = max(1, (nhi_max + PART - 1) // PART)
    KT = KLO + KHI
    CL = (KLO + CHUNK - 1) // CHUNK      # main-lo gather instructions
    CH = (KHI + CHUNK - 1) // CHUNK

    idx_lo = np.zeros((NCORES, NB, PART, KLO * 8), np.int16)
    idx_hi = np.zeros((NCORES, NB, PART, KHI * 8), np.int16)
    idx_allo = np.zeros((NCORES, NB, PART, KLO * 8), np.int16)
    idx_alhi = np.zeros((NCORES, NB, PART, KHI * 8), np.int16)
    dl_t = np.full((NCORES, NB, PART, KT), 999.0, np.float16)
    dm16_t = np.zeros((NCORES, NB, PART, KT), np.float16)
    cnt_t = np.zeros((NCORES, NB, 8), np.int32)
    for r in range(NCORES):
        for b in range(NB):
            lo_src, hi_src, abs_lo, abs_hi = blocks[r][b]
            ls, c_lo = _pad_stream(lo_src, KLO * PART)
            hs, c_hi = _pad_stream(hi_src, KHI * PART)
            al_lo, _ = _pad_stream(abs_lo // 16, KLO * PART)
            al_hi, _ = _pad_stream(abs_hi // 16, KHI * PART)
            idx_lo[r, b] = _pack16(ls, KLO * 8)
            idx_hi[r, b] = _pack16(hs, KHI * 8)
            idx_allo[r, b] = _pack16(al_lo, KLO * 8)
            idx_alhi[r, b] = _pack16(al_hi, KHI * 8)
            cnt_t[r, b, :CL] = c_lo
            cnt_t[r, b, CL:CL + CH] = c_hi
            # dst-local-within-block and abs%16 planes, stream position
            # i -> [i%128, i//128]
            dl = np.full(KT * PART, 999.0, np.float32)
            dm = np.zeros(KT * PART, np.float32)
            dl[:len(abs_lo)] = (abs_lo - r * NPC) % PART
            dm[:len(abs_lo)] = abs_lo % 16
            dl[KLO * PART:KLO * PART + len(abs_hi)] = (abs_hi - r * NPC) % PART
            dm[KLO * PART:KLO * PART + len(abs_hi)] = abs_hi % 16
            dl_t[r, b] = dl.reshape(KT, PART).T.astype(np.float16)
            dm16_t[r, b] = dm.reshape(KT, PART).T.astype(np.float16)

    # ---- batch-derived pooling metadata ----
    counts = np.bincount(batch, minlength=G).astype(np.float64)
    maskrow = np.zeros((NCORES, 1, NPC), np.float32)
    cinvrow = np.zeros((NCORES, 1, NPC), np.float32)
    lastcol = [dict() for _ in range(NCORES)]  # graph -> last own column
    for r in range(NCORES):
        bseg = batch[r * NPC:(r + 1) * NPC]
        same = np.ones(NPC, np.float32)
        same[0] = 0.0
        same[1:] = (bseg[1:] == bseg[:-1]).astype(np.float32)
        maskrow[r, 0] = same
        cinvrow[r, 0] = (1.0 / np.maximum(counts[bseg], 1.0)).astype(np.float32)
        gids, last_idx = np.unique(bseg[::-1], return_index=True)
        for g_, li in zip(gids, last_idx):
            lastcol[r][int(g_)] = NPC - 1 - int(li)

    # ---- weights (replicated) ----
    def bmat(W, a_s, a_d, fin):
        Wr = W.reshape(fin, H, C0)
        bs = np.einsum("khc,hc->kh", Wr, np.asarray(a_s, np.float32))
        bd = np.einsum("khc,hc->kh", Wr, np.asarray(a_d, np.float32))
        return np.concatenate([bs, bd], axis=1).astype(np.float16)  # [fin, 8]

    xh = np.zeros((F_IN, NPAD), np.float16)
    xh[:, :N] = x.T.astype(np.float16)

    shared = dict(
        xh16T=xh,
        W1h=W1.astype(np.float16), B1h=bmat(W1, att_src1, att_dst1, F_IN),
        W2h=W2.astype(np.float16), B2h=bmat(W2, att_src2, att_dst2, HC),
        b1row=np.asarray(b1, np.float32).reshape(2, PART),
        b2row=np.asarray(b2, np.float32).reshape(2, PART),
        gcol=np.asarray(gamma, np.float32).reshape(2, PART).T.copy(),
        bcol=np.asarray(beta, np.float32).reshape(2, PART).T.copy(),
    )
    in_maps = []
    for r in range(NCORES):
        in_maps.append(dict(
            shared,
            idx_lo=idx_lo[r], idx_hi=idx_hi[r],
            idx_allo=idx_allo[r], idx_alhi=idx_alhi[r],
            dl16=dl_t[r], dm16=dm16_t[r],
            cnts=cnt_t[r].reshape(1, NB * 8),
            maskrow=maskrow[r],
            cinvrow=cinvrow[r],
        ))
    meta = dict(NB=NB, KLO=KLO, KHI=KHI, KT=KT, CL=CL, CH=CH,
                lastcol=lastcol, counts=counts)
    return in_maps, meta


# --------------------------------------------------------------------------
# device program
# --------------------------------------------------------------------------

def build_program(meta, sim_local=False):
    NB, KLO, KHI, KT = meta["NB"], meta["KLO"], meta["KHI"], meta["KT"]
    CL, CH = meta["CL"], meta["CH"]
    nc = bacc.Bacc("TRN2", target_bir_lowering=False, debug=False,
                   num_devices=1 if sim_local else NCORES)

    def ein(name, shape, dt=F32):
        return nc.dram_tensor(name, list(shape), dt, kind="ExternalInput").ap()

    xh_d = ein("xh16T", [F_IN, NPAD], F16)
    W1_d = ein("W1h", [F_IN, HC], F16); B1_d = ein("B1h", [F_IN, 8], F16)
    W2_d = ein("W2h", [HC, HC], F16);   B2_d = ein("B2h", [HC, 8], F16)
    b1r_d = ein("b1row", [2, PART]); b2r_d = ein("b2row", [2, PART])
    gcol_d = ein("gcol", [PART, 2]); bcol_d = ein("bcol", [PART, 2])
    ilo_d = ein("idx_lo", [NB, PART, KLO * 8], I16)
    ihi_d = ein("idx_hi", [NB, PART, KHI * 8], I16)
    ialo_d = ein("idx_allo", [NB, PART, KLO * 8], I16)
    ialh_d = ein("idx_alhi", [NB, PART, KHI * 8], I16)
    dl_d = ein("dl16", [NB, PART, KT], F16)
    dm_d = ein("dm16", [NB, PART, KT], F16)
    cnt_d = ein("cnts", [1, NB * 8], I32)
    mask_d = ein("maskrow", [1, NPC])
    cinv_d = ein("cinvrow", [1, NPC])

    omax_d = nc.dram_tensor("out_max", [4 * PART, NPC], F32, kind="ExternalOutput").ap()
    omean_d = nc.dram_tensor("out_mean", [4 * PART, NPC], F32, kind="ExternalOutput").ap()

    # internal DRAM
    T1b = nc.dram_tensor("T1b", [NPAD, RWH], F16).ap()
    T2b = nc.dram_tensor("T2b", [NPAD, RWH], F16).ap()
    al1pk = nc.dram_tensor("al1pk", [NPAD // 16, 64], F32).ap()
    al2pk = nc.dram_tensor("al2pk", [NPAD // 16, 64], F32).ap()
    g1T = nc.dram_tensor("g1T", [HC, NPC], F32).ap()
    x2T = nc.dram_tensor("x2T", [HC, NPC], F32).ap()
    ar_in = nc.dram_tensor("ar_in", [PART, 4], F32).ap()
    ar_out = nc.dram_tensor("ar_out", [PART, 4], F32, addr_space="Shared").ap()

    # AllGather chunks of the inter-layer feature (transposed, fp16):
    # g1h_c [256, cols] per core -> Tag_c [8*256, cols]
    blk_of_chunk = []
    bpc = (NB + NAGC - 1) // NAGC
    for c in range(NAGC):
        blk_of_chunk.append(list(range(c * bpc, min(NB, (c + 1) * bpc))))
    chunk_cols = []
    g1h_c, Tag_c = [], []
    for c in range(NAGC):
        c0 = blk_of_chunk[c][0] * PART
        c1 = min(NPC, (blk_of_chunk[c][-1] + 1) * PART)
        chunk_cols.append((c0, c1))
        g1h_c.append(nc.dram_tensor(f"g1h_{c}", [HC, c1 - c0], F16).ap())
        Tag_c.append(nc.dram_tensor(f"Tag_{c}", [NCORES * HC, c1 - c0], F16,
                                    addr_space="Shared").ap())

    rgroups = [list(range(NCORES))]

    class _PhaseStopE(Exception):
        pass

    with tile.TileContext(nc) as tc:
      try:
        # ---------- shared constant tiles ----------
        with tc.tile_pool(name="const", bufs=1) as cpool:
            ident = cpool.tile([PART, PART], F32)
            make_identity(nc, ident[:])
            iota_i = cpool.tile([PART, PART], mybir.dt.int32)
            nc.gpsimd.iota(iota_i[:], pattern=[[1, PART]], base=0,
                           channel_multiplier=0)
            iota_h = cpool.tile([PART, PART], F16)
            nc.vector.tensor_copy(out=iota_h[:], in_=iota_i[:])
            iota16 = cpool.tile([PART, 16], F16)
            nc.vector.tensor_copy(out=iota16[:], in_=iota_i[:, 0:16])

            cnt_sb = cpool.tile([1, NB * 8], I32)
            nc.sync.dma_start(out=cnt_sb[:], in_=cnt_d[:, :])

            def bias_bcast(row_d, pool, psum_pool, tag):
                bv = pool.tile([PART, 2], F32, tag=f"biasv{tag}")
                nc.sync.dma_start(out=bv[:], in_=row_d[:, :].rearrange("c p -> p c"))
                bb = pool.tile([PART, HC], F32, tag=f"biasb{tag}")
                for c in range(2):
                    tp = psum_pool.tile([PART, PART], F32, tag=f"biastp{tag}{c}")
                    nc.tensor.transpose(out=tp[:], in_=bv[:, c:c + 1].to_broadcast([PART, PART]),
                                        identity=ident[:])
                    nc.vector.tensor_copy(out=bb[:, c * PART:(c + 1) * PART], in_=tp[:])
                return bb

            with tc.tile_pool(name="biasps", bufs=1, space="PSUM") as bps:
                b1b = bias_bcast(b1r_d, cpool, bps, "1")
                b2b = bias_bcast(b2r_d, cpool, bps, "2")

            # ---------- dense L1 (replicated: full table on every core) ----
            _sc = nc.enter_named_scope("dense1", False)[0]
            with tc.tile_pool(name="d1", bufs=3) as dp, \
                 tc.tile_pool(name="d1w", bufs=1) as wp, \
                 tc.tile_pool(name="d1x", bufs=2) as xp, \
                 tc.tile_pool(name="d1ps", bufs=2, space="PSUM") as pp:
                W1_sb = wp.tile([F_IN, HC], F16)
                nc.sync.dma_start(out=W1_sb[:], in_=W1_d[:, :])
                B1_sb = wp.tile([F_IN, 8], F16)
                nc.sync.dma_start(out=B1_sb[:], in_=B1_d[:, :])
                XCH = 6272                      # x column chunk (49 blocks)
                for b in range(NBLK):
                    if b % 49 == 0:
                        x_sb = xp.tile([F_IN, XCH], F16, tag="xsb")
                        x0 = b * PART
                        nc.sync.dma_start(out=x_sb[:, 0:min(XCH, NPAD - x0)],
                                          in_=xh_d[:, x0:min(x0 + XCH, NPAD)])
                    col = (b % 49) * PART
                    ps = pp.tile([PART, 264], F32, tag="dps")
                    nc.tensor.matmul(ps[:, 0:HC], lhsT=x_sb[:, col:col + PART],
                                     rhs=W1_sb[:], start=True, stop=True)
                    nc.tensor.matmul(ps[:, HC:HC + 8], lhsT=x_sb[:, col:col + PART],
                                     rhs=B1_sb[:], start=True, stop=True)
                    row = dp.tile([PART, RWH], F16, tag="drow")
                    nc.vector.tensor_copy(out=row[:, 0:HC], in_=ps[:, 0:HC])
                    nc.vector.tensor_copy(out=row[:, HC:HC + 8].bitcast(F32),
                                          in_=ps[:, HC:HC + 4])
                    nc.sync.dma_start(out=T1b[b * PART:(b + 1) * PART, :],
                                      in_=row[:])
                    alw = dp.tile([PART, 4], F32, tag="dal")
                    nc.vector.tensor_copy(out=alw[:], in_=ps[:, HC + 4:HC + 8])
                    nc.sync.dma_start(
                        out=al1pk[:, :].rearrange("r (g c) -> (r g) c", g=16)[
                            b * PART:(b + 1) * PART, :],
                        in_=alw[:])
            nc.leave_named_scope("dense1", _sc, False)

            # ---------- edge phase (shared for both layers) ----------
            def edge_phase(Tbl, alpk, bbias, outT, relu, scope, g1h_out=None):
                _es = nc.enter_named_scope(scope, False)[0]
                with tc.tile_pool(name="eidx", bufs=2) as ip, \
                     tc.tile_pool(name="eg", bufs=2) as gp, \
                     tc.tile_pool(name="ew", bufs=2) as wp2, \
                     tc.tile_pool(name="eps", bufs=2, space="PSUM") as ep, \
                     tc.tile_pool(name="etps", bufs=2, space="PSUM") as tps:
                    # pre-zero both gather buffers so -1-skipped slots always
                    # hold finite floats (first block would otherwise read
                    # uninitialized SBUF -> NaN * 0 = NaN in PSUM)
                    for _z in range(2):
                        for tg, shp, dt_ in (("glo", [PART, KLO, RWH], F16),
                                             ("ghi", [PART, KHI, RWH], F16),
                                             ("ga", [PART, KT, 64], F32)):
                            zt = gp.tile(shp, dt_, tag=tg)
                            nc.vector.memset(zt[:], 0.0)

                    cnt_regs = [nc.gpsimd.alloc_register(f"cnt_{scope}_{i}")
                                for i in range(4)]
                    reg_rr = [0]

                    def gather(gtile, src, ixtile, ktot, elem, cnt_base):
                        for ci, c0 in enumerate(range(0, ktot, CHUNK)):
                            cw = min(CHUNK, ktot - c0)
                            reg = cnt_regs[reg_rr[0] % 4]
                            reg_rr[0] += 1
                            nc.gpsimd.reg_load(
                                reg, cnt_sb[0:1, cnt_base + ci:cnt_base + ci + 1])
                            nc.gpsimd.dma_gather(
                                out_ap=gtile[:, c0:c0 + cw, :],
                                in_ap=src, idxs_ap=ixtile[:, c0 * 8:(c0 + cw) * 8],
                                num_idxs=cw * PART, num_idxs_reg=reg,
                                elem_size=elem)

                    for b in range(NB):
                        mb = min(PART, NPC - b * PART)
                        il = ip.tile([PART, KLO * 8], I16, tag="il")
                        nc.sync.dma_start(out=il[:], in_=ilo_d[b, :, :])
                        ih = ip.tile([PART, KHI * 8], I16, tag="ih")
                        nc.sync.dma_start(out=ih[:], in_=ihi_d[b, :, :])
                        ial = ip.tile([PART, KLO * 8], I16, tag="ial")
                        nc.sync.dma_start(out=ial[:], in_=ialo_d[b, :, :])
                        iah = ip.tile([PART, KHI * 8], I16, tag="iah")
                        nc.sync.dma_start(out=iah[:], in_=ialh_d[b, :, :])
                        dl = ip.tile([PART, KT], F16, tag="dl")
                        nc.sync.dma_start(out=dl[:], in_=dl_d[b, :, :])
                        dm = ip.tile([PART, KT], F16, tag="dm")
                        nc.sync.dma_start(out=dm[:], in_=dm_d[b, :, :])

                        glo = gp.tile([PART, KLO, RWH], F16, tag="glo")
                        gather(glo, Tbl[0:SPLIT, :], il, KLO, RWH, b * 8)
                        ghi = gp.tile([PART, KHI, RWH], F16, tag="ghi")
                        gather(ghi, Tbl[SPLIT:NPAD, :], ih, KHI, RWH, b * 8 + CL)
                        ga = gp.tile([PART, KT, 64], F32, tag="ga")
                        gather(ga[:, 0:KLO, :], alpk[:, :], ial, KLO, 64, b * 8)
                        gather(ga[:, KLO:KT, :], alpk[:, :], iah, KHI, 64,
                               b * 8 + CL)

                        # selector matrix S01[e, kt, d] = (dl == d), fp16
                        S01 = wp2.tile([PART, KT, PART], F16, tag="S01")
                        for e0 in range(0, KT, 4):
                            cnt = min(4, KT - e0)
                            nc.vector.tensor_tensor(
                                out=S01[:, e0:e0 + cnt, :],
                                in0=dl[:, e0:e0 + cnt].unsqueeze(-1).to_broadcast([PART, cnt, PART]),
                                in1=iota_h[:].unsqueeze(1).to_broadcast([PART, cnt, PART]),
                                op=ALU.is_equal)

                        # al_dst extraction: one-hot over the 16-node pack
                        oh = wp2.tile([PART, KT, 16], F32, tag="oh")
                        nc.vector.tensor_tensor(
                            out=oh[:],
                            in0=dm[:].unsqueeze(-1).to_broadcast([PART, KT, 16]),
                            in1=iota16[:].unsqueeze(1).to_broadcast([PART, KT, 16]),
                            op=ALU.is_equal)
                        adp = wp2.tile([PART, KT, 4, 16], F32, tag="adp")
                        nc.vector.tensor_tensor(
                            out=adp[:],
                            in0=ga[:].rearrange("p k (j h) -> p k h j", j=16),
                            in1=oh[:].unsqueeze(2).to_broadcast([PART, KT, 4, 16]),
                            op=ALU.mult)
                        Z = wp2.tile([PART, KT, 4], F32, tag="Z")
                        nc.vector.tensor_reduce(
                            out=Z[:].unsqueeze(-1), in_=adp[:],
                            axis=mybir.AxisListType.X, op=ALU.add)
                        # += al_src (f32 bits riding in the fp16 rows)
                        nc.vector.tensor_tensor(
                            out=Z[:, 0:KLO, :], in0=Z[:, 0:KLO, :],
                            in1=glo[:, :, HC:HC + 8].bitcast(F32), op=ALU.add)
                        nc.vector.tensor_tensor(
                            out=Z[:, KLO:KT, :], in0=Z[:, KLO:KT, :],
                            in1=ghi[:, :, HC:HC + 8].bitcast(F32), op=ALU.add)
                        # leaky-relu, clamp, exp -> fp16
                        ZT = wp2.tile([PART, KT, 4], F32, tag="ZT")
                        nc.vector.tensor_scalar_mul(out=ZT[:], in0=Z[:], scalar1=NEG_SLOPE)
                        nc.vector.tensor_tensor(out=Z[:], in0=Z[:], in1=ZT[:], op=ALU.max)
                        nc.vector.tensor_scalar_min(out=Z[:], in0=Z[:], scalar1=ZCLAMP)
                        EXh = wp2.tile([PART, KT, 4], F16, tag="EXh")
                        nc.scalar.activation(out=EXh[:], in_=Z[:], func=ACTF.Exp)

                        # Hp = [ex-weighted h | ex] (fp16)
                        Hp = wp2.tile([PART, KT, 260], F16, tag="Hp")
                        nc.vector.tensor_tensor(
                            out=Hp[:, 0:KLO, 0:HC].rearrange("p k (h c) -> p k h c", h=H),
                            in0=glo[:, :, 0:HC].rearrange("p k (h c) -> p k h c", h=H),
                            in1=EXh[:, 0:KLO, :].unsqueeze(-1).to_broadcast([PART, KLO, H, C0]),
                            op=ALU.mult)
                        nc.vector.tensor_tensor(
                            out=Hp[:, KLO:KT, 0:HC].rearrange("p k (h c) -> p k h c", h=H),
                            in0=ghi[:, :, 0:HC].rearrange("p k (h c) -> p k h c", h=H),
                            in1=EXh[:, KLO:KT, :].unsqueeze(-1).to_broadcast([PART, KHI, H, C0]),
                            op=ALU.mult)
                        nc.vector.tensor_copy(out=Hp[:, :, HC:HC + 4], in_=EXh[:])

                        acc = ep.tile([PART, 260], F32, tag="acc")
                        for e in range(KT):
                            nc.tensor.matmul(acc[:], lhsT=S01[:, e, :], rhs=Hp[:, e, :],
                                             start=(e == 0), stop=(e == KT - 1))

                        dn = wp2.tile([PART, 4], F32, tag="dn")
                        nc.vector.tensor_scalar_add(out=dn[:], in0=acc[:, HC:HC + 4],
                                                    scalar1=1e-16)
                        rec = wp2.tile([PART, 4], F32, tag="rec")
                        nc.vector.reciprocal(out=rec[:], in_=dn[:])
                        ob = wp2.tile([PART, HC], F32, tag="ob")
                        nc.vector.tensor_tensor(
                            out=ob[:].rearrange("p (h c) -> p h c", h=H),
                            in0=acc[:, 0:HC].rearrange("p (h c) -> p h c", h=H),
                            in1=rec[:].unsqueeze(-1).to_broadcast([PART, H, C0]),
                            op=ALU.mult)
                        nc.vector.tensor_tensor(out=ob[:], in0=ob[:], in1=bbias[:], op=ALU.add)
                        if relu:
                            nc.vector.tensor_scalar_max(out=ob[:], in0=ob[:], scalar1=0.0)
                        for c in range(2):
                            tp = tps.tile([PART, PART], F32, tag="ttp")
                            nc.tensor.transpose(out=tp[:], in_=ob[:, c * PART:(c + 1) * PART],
                                                identity=ident[:])
                            tsb = wp2.tile([PART, PART], F32, tag="tsb")
                            nc.vector.tensor_copy(out=tsb[:], in_=tp[:])
                            nc.sync.dma_start(
                                out=outT[c * PART:(c + 1) * PART, b * PART:b * PART + mb],
                                in_=tsb[:, 0:mb])
                            if g1h_out is not None:
                                tsh = wp2.tile([PART, PART], F16, tag="tsh")
                                nc.vector.tensor_copy(out=tsh[:], in_=tp[:])
                                ci, (cc0, _) = g1h_out(b)
                                nc.sync.dma_start(
                                    out=g1h_c[ci][c * PART:(c + 1) * PART,
                                                  b * PART - cc0:b * PART - cc0 + mb],
                                    in_=tsh[:, 0:mb])
                        # fire the AllGather chunk when its last block is done
                        if g1h_out is not None:
                            ci, _ = g1h_out(b)
                            if b == blk_of_chunk[ci][-1]:
                                if sim_local:
                                    for r_ in range(NCORES):
                                        nc.sync.dma_start(
                                            out=Tag_c[ci][r_ * HC:(r_ + 1) * HC, :],
                                            in_=g1h_c[ci][:, :])
                                else:
                                    nc.gpsimd.collective_compute(
                                        "AllGather", ALU.bypass,
                                        replica_groups=rgroups,
                                        ins=[g1h_c[ci][:, :]],
                                        outs=[Tag_c[ci][:, :]])
                nc.leave_named_scope(scope, _es, False)

            def _g1h_of(b):
                ci = min(b // bpc, NAGC - 1)
                return ci, chunk_cols[ci]

            if PHASES >= 2:
                edge_phase(T1b, al1pk, b1b, g1T, relu=False, scope="edge1",
                           g1h_out=_g1h_of)

            # ---------- BN stats + AllReduce ----------
            if PHASES < 3:
                raise _PhaseStopE
            _sc = nc.enter_named_scope("bnstat", False)[0]
            with tc.tile_pool(name="st", bufs=1) as sp, \
                 tc.tile_pool(name="stw", bufs=1) as sw:
                stats = sw.tile([PART, 4], F32)
                for ct in range(2):
                    gt = sp.tile([PART, NPC], F32, tag="gt")
                    nc.sync.dma_start(out=gt[:], in_=g1T[ct * PART:(ct + 1) * PART, :])
                    nc.vector.tensor_reduce(out=stats[:, ct:ct + 1], in_=gt[:],
                                            axis=mybir.AxisListType.X, op=ALU.add)
                    sq = sp.tile([PART, NPC], F32, tag="sq")
                    nc.scalar.activation(out=sq[:], in_=gt[:], func=ACTF.Square)
                    nc.vector.tensor_reduce(out=stats[:, 2 + ct:3 + ct], in_=sq[:],
                                            axis=mybir.AxisListType.X, op=ALU.add)
                nc.sync.dma_start(out=ar_in[:, :], in_=stats[:])
            nc.leave_named_scope("bnstat", _sc, False)

            _sc = nc.enter_named_scope("ar", False)[0]
            if sim_local:
                nc.sync.dma_start(out=ar_out[:, :], in_=ar_in[:, :])
            else:
                nc.gpsimd.collective_compute(
                    "AllReduce", ALU.add, replica_groups=rgroups,
                    ins=[ar_in[:, :]], outs=[ar_out[:, :]])
            nc.leave_named_scope("ar", _sc, False)

            with tc.tile_pool(name="bnw", bufs=1) as bw:
                ar_sb = bw.tile([PART, 4], F32)
                nc.sync.dma_start(out=ar_sb[:], in_=ar_out[:, :])
                mean = bw.tile([PART, 2], F32)
                nc.vector.tensor_scalar_mul(out=mean[:], in0=ar_sb[:, 0:2], scalar1=1.0 / N)
                msq = bw.tile([PART, 2], F32)
                nc.vector.tensor_scalar_mul(out=msq[:], in0=ar_sb[:, 2:4], scalar1=1.0 / N)
                var = bw.tile([PART, 2], F32)
                nc.vector.tensor_tensor(out=var[:], in0=mean[:], in1=mean[:], op=ALU.mult)
                nc.vector.tensor_tensor(out=var[:], in0=msq[:], in1=var[:], op=ALU.subtract)
                nc.vector.tensor_scalar_add(out=var[:], in0=var[:], scalar1=BN_EPS)
                sd = bw.tile([PART, 2], F32)
                nc.scalar.activation(out=sd[:], in_=var[:], func=ACTF.Sqrt)
                rinv = bw.tile([PART, 2], F32)
                nc.vector.reciprocal(out=rinv[:], in_=sd[:])
                gc = bw.tile([PART, 2], F32)
                nc.sync.dma_start(out=gc[:], in_=gcol_d[:, :])
                bc = bw.tile([PART, 2], F32)
                nc.sync.dma_start(out=bc[:], in_=bcol_d[:, :])
                scale_c = bw.tile([PART, 2], F32)
                nc.vector.tensor_tensor(out=scale_c[:], in0=gc[:], in1=rinv[:], op=ALU.mult)
                shift_c = bw.tile([PART, 2], F32)
                nc.vector.tensor_tensor(out=shift_c[:], in0=mean[:], in1=scale_c[:], op=ALU.mult)
                nc.vector.tensor_tensor(out=shift_c[:], in0=bc[:], in1=shift_c[:], op=ALU.subtract)

                # ---------- dense L2 (replicated, from AllGathered x1) -----
                if PHASES < 4:
                    raise _PhaseStopE
                _sc = nc.enter_named_scope("dense2", False)[0]
                with tc.tile_pool(name="d2", bufs=3) as dp2, \
                     tc.tile_pool(name="d2w", bufs=1) as wp3, \
                     tc.tile_pool(name="d2x", bufs=2) as xp2, \
                     tc.tile_pool(name="d2ps", bufs=2, space="PSUM") as pp2:
                    W2_sb = [wp3.tile([PART, HC], F16, tag=f"w2_{kt}", name=f"w2_{kt}")
                             for kt in range(2)]
                    B2_sb = [wp3.tile([PART, 8], F16, tag=f"b2_{kt}", name=f"b2_{kt}")
                             for kt in range(2)]
                    for kt in range(2):
                        nc.sync.dma_start(out=W2_sb[kt][:],
                                          in_=W2_d[kt * PART:(kt + 1) * PART, :])
                        nc.sync.dma_start(out=B2_sb[kt][:],
                                          in_=B2_d[kt * PART:(kt + 1) * PART, :])
                    gblk = 0       # global dense block counter (over NPAD)
                    for r_ in range(NCORES):
                        for ci in range(NAGC):
                            cc0, cc1 = chunk_cols[ci]
                            w = cc1 - cc0
                            xs = []
                            for kt in range(2):
                                gl = xp2.tile([PART, bpc * PART], F16, tag=f"gl{kt}")
                                nc.sync.dma_start(
                                    out=gl[:, 0:w],
                                    in_=Tag_c[ci][r_ * HC + kt * PART:r_ * HC + (kt + 1) * PART, :])
                                x1s = xp2.tile([PART, bpc * PART], F16, tag=f"x1s{kt}")
                                nc.scalar.activation(out=x1s[:, 0:w], in_=gl[:, 0:w],
                                                     func=ACTF.Relu,
                                                     bias=shift_c[:, kt:kt + 1],
                                                     scale=scale_c[:, kt:kt + 1])
                                xs.append(x1s)
                            for lb in range(0, w, PART):
                                mb2 = min(PART, w - lb)
                                ps = pp2.tile([PART, 264], F32, tag="d2ps")
                                for kt in range(2):
                                    nc.tensor.matmul(ps[0:mb2, 0:HC],
                                                     lhsT=xs[kt][:, lb:lb + mb2],
                                                     rhs=W2_sb[kt][:],
                                                     start=(kt == 0), stop=(kt == 1))
                                for kt in range(2):
                                    nc.tensor.matmul(ps[0:mb2, HC:HC + 8],
                                                     lhsT=xs[kt][:, lb:lb + mb2],
                                                     rhs=B2_sb[kt][:],
                                                     start=(kt == 0), stop=(kt == 1))
                                row = dp2.tile([PART, RWH], F16, tag="d2row")
                                nc.vector.tensor_copy(out=row[0:mb2, 0:HC], in_=ps[0:mb2, 0:HC])
                                nc.vector.tensor_copy(out=row[0:mb2, HC:HC + 8].bitcast(F32),
                                                      in_=ps[0:mb2, HC:HC + 4])
                                n0 = r_ * NPC + cc0 + lb
                                nc.sync.dma_start(out=T2b[n0:n0 + mb2, :],
                                                  in_=row[0:mb2, :])
                                alw = dp2.tile([PART, 4], F32, tag="d2al")
                                nc.vector.tensor_copy(out=alw[0:mb2, :],
                                                      in_=ps[0:mb2, HC + 4:HC + 8])
                                nc.sync.dma_start(
                                    out=al2pk[:, :].rearrange(
                                        "r (g c) -> (r g) c", g=16)[n0:n0 + mb2, :],
                                    in_=alw[0:mb2, :])
                nc.leave_named_scope("dense2", _sc, False)

                if PHASES < 5:
                    raise _PhaseStopE
                edge_phase(T2b, al2pk, b2b, x2T, relu=True, scope="edge2")

                # ---------- pooling ----------
                if PHASES < 6:
                    raise _PhaseStopE
                _sc = nc.enter_named_scope("pool", False)[0]
                with tc.tile_pool(name="pl", bufs=1) as pl:
                    mk = pl.tile([PART, NPC], F32, tag="mk")
                    nc.sync.dma_start(out=mk[:], in_=mask_d[0:1, :].to_broadcast([PART, NPC]))
                    cv = pl.tile([PART, NPC], F32, tag="cv")
                    nc.sync.dma_start(out=cv[:], in_=cinv_d[0:1, :].to_broadcast([PART, NPC]))
                    for ct in range(4):
                        xt = pl.tile([PART, NPC], F32, tag="xt")
                        if ct < 2:
                            gld = pl.tile([PART, NPC], F32, tag="gld")
                            nc.sync.dma_start(out=gld[:], in_=g1T[ct * PART:(ct + 1) * PART, :])
                            nc.scalar.activation(out=xt[:], in_=gld[:], func=ACTF.Relu,
                                                 bias=shift_c[:, ct:ct + 1],
                                                 scale=scale_c[:, ct:ct + 1])
                        else:
                            nc.sync.dma_start(out=xt[:],
                                              in_=x2T[(ct - 2) * PART:(ct - 1) * PART, :])
                        sm = pl.tile([PART, NPC], F32, tag="sm")
                        nc.vector.tensor_tensor_scan(out=sm[:], data0=mk[:], data1=xt[:],
                                                     initial=0.0, op0=ALU.mult, op1=ALU.max)
                        nc.sync.dma_start(out=omax_d[ct * PART:(ct + 1) * PART, :], in_=sm[:])
                        ss = pl.tile([PART, NPC], F32, tag="ss")
                        nc.vector.tensor_tensor_scan(out=ss[:], data0=mk[:], data1=xt[:],
                                                     initial=0.0, op0=ALU.mult, op1=ALU.add)
                        nc.vector.tensor_tensor(out=ss[:], in0=ss[:], in1=cv[:], op=ALU.mult)
                        nc.sync.dma_start(out=omean_d[ct * PART:(ct + 1) * PART, :], in_=ss[:])
                nc.leave_named_scope("pool", _sc, False)

      except _PhaseStopE:
        pass

    nc.compile()
    return nc


# --------------------------------------------------------------------------
# host-side combine
# --------------------------------------------------------------------------

def postprocess(results, meta):
    lastcol = meta["lastcol"]
    mean = np.zeros((G, 2 * HC), np.float32)
    mx = np.zeros((G, 2 * HC), np.float32)
    for r in range(NCORES):
        om = results[r]["out_mean"]   # [512, NPC]
        ox = results[r]["out_max"]
        for g_, col in lastcol[r].items():
            mean[g_] += om[:, col]
            mx[g_] = np.maximum(mx[g_], ox[:, col])
    # empty graphs stay 0 (matches reference semantics)
    return np.concatenate([mean, mx], axis=1).astype(np.float32)


_CACHE = {}


def kernel(**inputs):
    in_maps, meta = preprocess(**inputs)
    key = (meta["NB"], meta["KLO"], meta["KHI"])
    if key not in _CACHE:
        _CACHE[key] = build_program(meta)
    nc = _CACHE[key]
    res = bass_utils.run_bass_kernel_spmd(nc, in_maps, core_ids=list(range(NCORES)))
    return postprocess(res.results, meta)
